# revision 2
# baseline (speedup 1.0000x reference)
"""Trainium2 Bass kernel for a 2-layer autoregressive LSTM (teacher-forced)
with zoneout (eval mode), conditioning input, and output projection.

Strategy (8 NeuronCores, one TRN2 chip):
  - Model-parallel over the 4*H=4096 gate dimension: core k owns hidden units
    [k*128, (k+1)*128) of each gate (i, f, o, g) for BOTH layers, full batch.
  - Per timestep each core computes its 512 gate rows with gate-stationary
    [128,128] matmul tiles (rhs = h^T [128, 32]), the LSTM cell elementwise on
    [128, 32] tiles, then all cores AllGather their 128-unit h slices (both
    layers fused in one 8KB payload) so everyone has the full h next step.
  - Layer 1 lags layer 0 by LAG=2L steps; h0/h1 share one history ring
    [P, KH+1, 2L, 2B] so a single DMA per step lands the gathered slices.
  - Input-side products are hoisted off the serial chain and batched over
    L-step chunks: U0 = xin @ W_ih0^T, U1 = h0 @ W_ih1^T, y = h1 @ proj^T,
    with emissions spread one m-tile per iteration to bound PE interference.
    U terms enter the per-step PSUM accumulation through an identity-weight
    matmul issued before the recurrent chunks; biases ride as ones-rows.
"""

import numpy as np

import concourse.bass as bass
import concourse.bacc as bacc
import concourse.tile as tile
from concourse import bass_utils, mybir

AF = mybir.ActivationFunctionType
ALU = mybir.AluOpType

# Problem constants
B, D, T_FULL, C, H = 32, 80, 1000, 512, 1024
ZONEOUT = 0.1

# Kernel layout constants
P = 128          # partitions
NC = 8           # cores
HU = H // NC     # hidden units per core = 128
MT = 4           # gate m-tiles per core (i, f, o, g)
KIN = 5          # xin contraction chunks (592+bias padded to 640 = 5*128)
KH = H // P      # h contraction chunks = 8
L = 16           # time-chunk length for the batched GEMMs
LAG = 2 * L      # layer-1 lag behind layer 0 (= ring size for slot reuse)
NU = 3           # U chunk buffers
PJ = D // NC     # proj rows per core = 10
PJP = 16         # padded proj rows per core

BF16 = mybir.dt.bfloat16
F32 = mybir.dt.float32
NP_BF16 = mybir.dt.np(BF16)

RG = [list(range(NC))]


def _chunks(T):
    n = (T + L - 1) // L
    return [(c, min(L, T - c * L)) for c in range(n)]


def build_nc(T):
    """Build the SPMD Bass program for sequence length T."""
    TB = T * B
    nc = bacc.Bacc(
        "TRN2",
        target_bir_lowering=False,
        debug=False,
        enable_asserts=False,
        num_devices=NC,
    )

    # ---- I/O ----
    xinT_d = nc.dram_tensor("xinT", [P, KIN, TB], BF16, kind="ExternalInput")
    w0T_d = nc.dram_tensor("w0T", [P, KIN, MT, P], BF16, kind="ExternalInput")
    wh0T_d = nc.dram_tensor("wh0T", [P, KH, MT, P], BF16, kind="ExternalInput")
    w1T_d = nc.dram_tensor("w1T", [P, KH + 1, MT, P], BF16, kind="ExternalInput")
    wh1T_d = nc.dram_tensor("wh1T", [P, KH, MT, P], BF16, kind="ExternalInput")
    pjT_d = nc.dram_tensor("pjT", [P, KH + 1, PJP], BF16, kind="ExternalInput")
    id_d = nc.dram_tensor("ident", [P, P], BF16, kind="ExternalInput")
    y_d = nc.dram_tensor("y_out", [PJP, TB], F32, kind="ExternalOutput")

    ch = _chunks(T)
    # emission schedule: iteration t -> list of (kind, chunk, m)
    sched = {}

    def _at(t, item):
        sched.setdefault(t, []).append(item)

    for c, lc in ch:
        if c >= 1:
            _at(max(0, (c - 1) * L - 1), ("x", c, 0))  # xin DMA one step early
            for m in range(MT):
                _at((c - 1) * L + m, ("u0", c, m))
        for m in range(MT):
            _at(c * L + lc + m, ("u1", c, m))
        _at(c * L + lc + LAG, ("pj", c, 0))

    with tile.TileContext(nc) as tc:
        with (
            tc.tile_pool(name="const", bufs=1) as cp,
            tc.tile_pool(name="work", bufs=3) as wp,
            tc.tile_pool(name="dram", bufs=3, space="DRAM") as dp,
            tc.tile_pool(name="ps0", bufs=2, space="PSUM") as ps0p,
            tc.tile_pool(name="ps1", bufs=2, space="PSUM") as ps1p,
            tc.tile_pool(name="psu", bufs=2, space="PSUM") as psup,
            tc.tile_pool(name="psp", bufs=1, space="PSUM") as pspp,
        ):
            # resident tiles
            w0_sb = cp.tile([P, KIN, MT, P], BF16)
            wh0_sb = cp.tile([P, KH, MT, P], BF16)
            w1_sb = cp.tile([P, KH + 1, MT, P], BF16)
            wh1_sb = cp.tile([P, KH, MT, P], BF16)
            pj_sb = cp.tile([P, KH + 1, PJP], BF16)
            id_sb = cp.tile([P, P], BF16)
            hist = cp.tile([P, KH + 1, 2 * L, 2 * B], BF16)
            U0_sb = cp.tile([P, NU, MT, L * B], BF16)
            U1_sb = cp.tile([P, NU, MT, L * B], BF16)
            c0_t = cp.tile([P, B], F32)
            c1_t = cp.tile([P, B], F32)

            nc.sync.dma_start(w0_sb[:], w0T_d[:])
            nc.sync.dma_start(wh0_sb[:], wh0T_d[:])
            nc.sync.dma_start(w1_sb[:], w1T_d[:])
            nc.sync.dma_start(wh1_sb[:], wh1T_d[:])
            nc.sync.dma_start(pj_sb[:], pjT_d[:])
            nc.sync.dma_start(id_sb[:], id_d[:])

            nc.vector.memset(hist[:], 0.0)
            nc.vector.memset(hist[:, KH, :, :], 1.0)  # bias ones-row block
            nc.vector.memset(c0_t[:], 0.0)
            nc.vector.memset(c1_t[:], 0.0)

            # h state (both layers, bf16): doubles as the AllGather payload.
            # Cycled through a pool so the send DMA of step t-1 never blocks
            # the writes of step t.
            hb_prev = wp.tile([P, 2 * B], BF16, tag="hb")
            nc.vector.memset(hb_prev[:], 0.0)

            def emit_xin(c):
                lc = ch[c][1]
                nco = lc * B
                xin_t = wp.tile([P, KIN, L * B], BF16, tag="xin")
                nc.sync.dma_start(
                    xin_t[:, :, :nco], xinT_d[:, :, c * L * B : c * L * B + nco]
                )
                return xin_t

            xin_cur = {}

            def emit_u0(c, m):
                lc = ch[c][1]
                nco = lc * B
                xin_t = xin_cur[c]
                pt = psup.tile([P, L * B], F32, tag="psu")
                for k in range(KIN):
                    nc.tensor.matmul(
                        pt[:, :nco],
                        w0_sb[:, k, m, :],
                        xin_t[:, k, :nco],
                        start=(k == 0),
                        stop=(k == KIN - 1),
                    )
                nc.vector.tensor_copy(U0_sb[:, c % NU, m, :nco], pt[:, :nco])

            def emit_u1(c, m):
                lc = ch[c][1]
                nco = lc * B
                half = (c % 2) * L
                pt = psup.tile([P, L * B], F32, tag="psu")
                for k in range(KH + 1):
                    rhs = hist[:, k, half : half + lc, 0:B]
                    nc.tensor.matmul(
                        pt[:, :nco],
                        w1_sb[:, k, m, :],
                        rhs,
                        start=(k == 0),
                        stop=(k == KH),
                    )
                nc.vector.tensor_copy(U1_sb[:, c % NU, m, :nco], pt[:, :nco])

            def emit_proj(c):
                lc = ch[c][1]
                nco = lc * B
                half = (c % 2) * L
                pt = pspp.tile([PJP, L * B], F32, tag="psp")
                for k in range(KH + 1):
                    rhs = hist[:, k, half : half + lc, B : 2 * B]
                    nc.tensor.matmul(
                        pt[:, :nco],
                        pj_sb[:, k, :],
                        rhs,
                        start=(k == 0),
                        stop=(k == KH),
                    )
                y_t = wp.tile([PJP, L * B], F32, tag="ysb")
                nc.scalar.copy(y_t[:, :nco], pt[:, :nco])
                nc.sync.dma_start(y_d[:, c * L * B : c * L * B + nco], y_t[:, :nco])

            def cell_mm_prep(ell, t):
                """Allocate PSUM + off-chain c01 for one cell step."""
                cst = c0_t if ell == 0 else c1_t
                pool = ps0p if ell == 0 else ps1p
                ps = pool.tile([P, MT * B], F32, tag=f"ps{ell}")
                c01 = wp.tile([P, B], F32, tag=f"c01{ell}")
                nc.vector.tensor_scalar_mul(c01[:], cst[:], ZONEOUT)
                return ps, c01, cst

            def cell_mm_tile(ell, t, mm, m):
                """One m-tile accumulation group of a cell's gate matmul."""
                ps, c01, cst = mm
                W = wh0_sb if ell == 0 else wh1_sb
                U = U0_sb if ell == 0 else U1_sb
                # slot (t-1) of the unified ring holds h(t-1) for this layer
                slot = (t - 1 + LAG if ell == 0 else t - 1 + 2 * LAG) % (2 * L)
                ci, si = t // L, t % L
                co = ell * B
                o = ps[:, m * B : (m + 1) * B]
                # U injection first: no dependence on the fresh hist DMA
                nc.tensor.matmul(
                    o, id_sb[:],
                    U[:, ci % NU, m, si * B : (si + 1) * B],
                    start=True, stop=False,
                )
                for k in range(KH):
                    nc.tensor.matmul(
                        o, W[:, k, m, :],
                        hist[:, k, slot, co : co + B],
                        start=False, stop=(k == KH - 1),
                    )

            def cell_post(ell, t, mm, hb, hbp):
                """ACT/DVE half: gates -> new h (bf16, into hb) and c."""
                ps, c01, cst = mm
                co = ell * B
                S = wp.tile([P, 3 * B], F32, tag=f"S{ell}")
                nc.scalar.activation(S[:], ps[:, 0 : 3 * B], AF.Sigmoid)
                Tg = wp.tile([P, B], F32, tag=f"Tg{ell}")
                nc.scalar.activation(Tg[:], ps[:, 3 * B : 4 * B], AF.Tanh)
                R = wp.tile([P, B], F32, tag=f"R{ell}")
                nc.vector.tensor_mul(R[:], S[:, B : 2 * B], cst[:])
                Pi = wp.tile([P, B], F32, tag=f"Pi{ell}")
                nc.vector.tensor_mul(Pi[:], S[:, 0:B], Tg[:])
                cn = wp.tile([P, B], F32, tag=f"cn{ell}")
                nc.vector.tensor_add(cn[:], R[:], Pi[:])
                # c <- 0.9*c_new + 0.1*c_old  (c01 precomputed off-chain)
                nc.vector.scalar_tensor_tensor(
                    cst[:], cn[:], 1.0 - ZONEOUT, c01[:], op0=ALU.mult, op1=ALU.add
                )
                Tc = wp.tile([P, B], F32, tag=f"Tc{ell}")
                nc.scalar.activation(Tc[:], cn[:], AF.Tanh)
                So9 = wp.tile([P, B], F32, tag=f"So9{ell}")
                nc.vector.tensor_scalar_mul(So9[:], S[:, 2 * B : 3 * B], 1.0 - ZONEOUT)
                Hz = wp.tile([P, B], F32, tag=f"Hz{ell}")
                nc.vector.tensor_mul(Hz[:], So9[:], Tc[:])
                # h <- 0.9*o*tanh(c_new) + 0.1*h_old, written bf16 into the
                # combined send/state tile
                nc.vector.scalar_tensor_tensor(
                    hb[:, co : co + B], hbp[:, co : co + B], ZONEOUT, Hz[:],
                    op0=ALU.mult, op1=ALU.add,
                )

            # chunk 0 prologue
            xin_cur[0] = emit_xin(0)
            for m in range(MT):
                emit_u0(0, m)

            for t in range(T + LAG):
                hb = wp.tile([P, 2 * B], BF16, tag="hb")
                tau = t - LAG
                mm0 = cell_mm_prep(0, t) if t < T else None
                mm1 = cell_mm_prep(1, tau) if tau >= 0 else None
                for m in range(MT):
                    if mm0 is not None:
                        cell_mm_tile(0, t, mm0, m)
                    if mm1 is not None:
                        cell_mm_tile(1, tau, mm1, m)
                if t < T:
                    cell_post(0, t, mm0, hb, hb_prev)
                else:
                    nc.vector.memset(hb[:, 0:B], 0.0)
                if tau >= 0:
                    cell_post(1, tau, mm1, hb, hb_prev)
                else:
                    nc.vector.memset(hb[:, B : 2 * B], 0.0)
                hb_prev = hb

                agi = dp.tile([P, 2 * B], BF16, tag="agi")
                ago = dp.tile([NC * P, 2 * B], BF16, tag="ago")
                nc.scalar.dma_start(agi[:], hb[:])
                nc.gpsimd.collective_compute(
                    "AllGather",
                    ALU.bypass,
                    replica_groups=RG,
                    ins=[agi.opt()],
                    outs=[ago.opt()],
                )
                agov = ago[:].rearrange("(k p) b -> p k b", p=P)
                nc.sync.dma_start(hist[:, 0:KH, t % (2 * L), :], agov[:])

                for kind, c, m in sched.get(t, ()):
                    if kind == "x":
                        xin_cur[c] = emit_xin(c)
                    elif kind == "u0":
                        emit_u0(c, m)
                    elif kind == "u1":
                        emit_u1(c, m)
                    elif kind == "pj":
                        emit_proj(c)

            # chunks scheduled past the last iteration
            for t_late in sorted(sched):
                if t_late >= T + LAG:
                    for kind, c, m in sched[t_late]:
                        if kind == "pj":
                            emit_proj(c)

    nc.compile()
    return nc


# ---------------- host-side data prep ----------------

def _gate_rows(k):
    u = np.arange(k * HU, (k + 1) * HU)
    return np.concatenate([u, H + u, 3 * H + u, 2 * H + u])  # i, f, o, g


def _lhsT_blocks(w, nk, mt=MT):
    """w: [mt*P, nk*P] (already row-sliced/ordered) -> [P, nk, mt, P] lhsT tiles."""
    a = w.reshape(mt, P, nk, P)  # [m, j, k, p]
    return np.ascontiguousarray(a.transpose(3, 2, 0, 1))  # [p, k, m, j]


def prep_inputs(inputs, T):
    x = np.asarray(inputs["x"], np.float32)[:, :, :T]
    cond = np.asarray(inputs["cond"], np.float32)[:, :, :T]
    w_ih0 = np.asarray(inputs["w_ih0"], np.float32)
    w_hh0 = np.asarray(inputs["w_hh0"], np.float32)
    b0 = np.asarray(inputs["b_ih0"], np.float32) + np.asarray(inputs["b_hh0"], np.float32)
    w_ih1 = np.asarray(inputs["w_ih1"], np.float32)
    w_hh1 = np.asarray(inputs["w_hh1"], np.float32)
    b1 = np.asarray(inputs["b_ih1"], np.float32) + np.asarray(inputs["b_hh1"], np.float32)
    proj_w = np.asarray(inputs["proj_w"], np.float32)
    proj_b = np.asarray(inputs["proj_b"], np.float32)

    TB = T * B
    in0 = D + C
    xs = np.concatenate([np.zeros((B, D, 1), np.float32), x[:, :, : T - 1]], axis=2)
    xin = np.concatenate([xs, cond], axis=1)  # [B, 592, T]
    xin_pad = np.zeros((B, KIN * P, T), np.float32)
    xin_pad[:, :in0] = xin
    xin_pad[:, in0] = 1.0  # bias feature
    # [feat, T, B] -> [feat, TB] with col index t*B+b
    xinT = np.ascontiguousarray(xin_pad.transpose(1, 2, 0)).reshape(KIN * P, TB)
    xinT = np.ascontiguousarray(
        xinT.reshape(KIN, P, TB).transpose(1, 0, 2)
    ).astype(NP_BF16)

    w_ih0_pad = np.zeros((4 * H, KIN * P), np.float32)
    w_ih0_pad[:, :in0] = w_ih0
    w_ih0_pad[:, in0] = b0

    ident = np.eye(P, dtype=NP_BF16)

    in_maps = []
    for k in range(NC):
        r = _gate_rows(k)
        w0T = _lhsT_blocks(w_ih0_pad[r], KIN).astype(NP_BF16)
        wh0T = _lhsT_blocks(w_hh0[r], KH).astype(NP_BF16)
        w1_ext = np.zeros((MT * P, (KH + 1) * P), np.float32)
        w1_ext[:, : KH * P] = w_ih1[r]
        w1_ext[:, KH * P] = b1[r]  # ones-row bias block (row 0 of chunk KH)
        w1T = _lhsT_blocks(w1_ext, KH + 1).astype(NP_BF16)
        wh1T = _lhsT_blocks(w_hh1[r], KH).astype(NP_BF16)
        pjT = np.zeros((P, KH + 1, PJP), np.float32)
        rows = np.arange(k * PJ, (k + 1) * PJ)
        for kk in range(KH):
            pjT[:, kk, :PJ] = proj_w[rows, kk * P : (kk + 1) * P].T
        pjT[0, KH, :PJ] = proj_b[rows]
        in_maps.append(
            {
                "xinT": xinT,
                "w0T": w0T,
                "wh0T": wh0T,
                "w1T": w1T,
                "wh1T": wh1T,
                "pjT": pjT.astype(NP_BF16),
                "ident": ident,
            }
        )
    return in_maps


def assemble(results, x_lengths, T):
    y = np.concatenate([r["y_out"][:PJ] for r in results], axis=0)  # [80, TB]
    y = y.reshape(D, T, B).transpose(2, 0, 1)  # [B, D, T]
    lens = np.asarray(x_lengths).astype(np.int64)
    mask = (np.arange(T)[None, :] < lens[:, None]).astype(np.float32)
    return np.ascontiguousarray(y * mask[:, None, :])


_NC_CACHE = {}


def run(inputs, T=T_FULL, trace=False, **kw):
    if T not in _NC_CACHE:
        _NC_CACHE[T] = build_nc(T)
    nc = _NC_CACHE[T]
    in_maps = prep_inputs(inputs, T)
    res = bass_utils.run_bass_kernel_spmd(
        nc, in_maps, core_ids=list(range(NC)), trace=trace, **kw
    )
    out = assemble(res.results, inputs["x_lengths"], T)
    return out, res


def kernel(**inputs) -> np.ndarray:
    out, _ = run(inputs, T=T_FULL)
    return out
